# revision 1
# baseline (speedup 1.0000x reference)
"""Causal multi-head attention (B=2, S=2048, D=1024, H=16, HD=64) on 8 NeuronCores.

Sharding: core c = 4*b + g handles batch b (2-way data parallel) and head
group g (4-way tensor parallel over the 16 heads, 4 heads per core).
Each core computes its 4 heads' attention plus the partial output
projection (columns of Wo for its heads); the host sums the 4 partials
per batch ("row-parallel" reduction) to produce the full output.

Device layout notes:
  - X is fed transposed (xt = X[b].T, [D,S]) so the d-contraction of the
    QKV projections has d on SBUF partitions.
  - Q,K are produced transposed ([d_head, s]); scores are computed
    transposed (S^T[k,q]) so the P@V matmul needs no transposes at all.
  - V is produced in natural [s, d] layout, augmented with a ones column
    per head so the P@V matmul also yields the softmax denominator.
  - softmax skips max-subtraction (scores/8 ~ N(0,1); exp is safe in f32).
  - matmuls run in float32r (TF32-like full-rate mode); the softmax
    probabilities and V run in fp16 (11-bit mantissa; P in [0,403]).
  - chunks 0 and 1 of the attention run inside the DMA-bound projection
    window (the scalar engine is idle there); the exp-heavy chunks 3 and 2
    run interleaved afterwards.
"""

import numpy as np

import concourse.mybir as mybir
from concourse import bacc
from concourse.tile import TileContext
from concourse.bass_utils import run_bass_kernel_spmd
from concourse.masks import make_upper_triangular

F32 = mybir.dt.float32
F32R = mybir.dt.float32r
FP16 = mybir.dt.float16
Exp = mybir.ActivationFunctionType.Exp
Alu = mybir.AluOpType

B, S, D, H, HD = 2, 2048, 1024, 16, 64
GH = 4            # heads per core
GD = GH * HD      # 256 features per core
N_CORES = 8


def _build():
    nc = bacc.Bacc("TRN2", target_bir_lowering=False, name="mha_tp")
    xt_d = nc.dram_tensor("xt", [D, S], F32R, kind="ExternalInput")
    wq_d = nc.dram_tensor("wqT", [D, GD], F32R, kind="ExternalInput")
    wk_d = nc.dram_tensor("wkT", [D, GD], F32R, kind="ExternalInput")
    wv_d = nc.dram_tensor("wvT", [D, GD], F32R, kind="ExternalInput")
    wo_d = nc.dram_tensor("woT", [GD, D], F32R, kind="ExternalInput")
    out_d = nc.dram_tensor("out", [S, D], F32, kind="ExternalOutput")

    with TileContext(nc) as tc:
        with (
            tc.tile_pool(name="per", bufs=1) as per,
            tc.tile_pool(name="pt", bufs=10) as ptp,
            tc.tile_pool(name="wk1", bufs=1) as wk1,
            tc.tile_pool(name="wk2", bufs=6) as wk2,
            tc.tile_pool(name="ps_a", bufs=2, space="PSUM") as ps_a,
            tc.tile_pool(name="ps_o", bufs=2, space="PSUM") as ps_o,
            tc.tile_pool(name="ps_c", bufs=2, space="PSUM") as ps_c,
        ):
            xt = per.tile([128, 8, S], F32R)       # X^T, d-tile major
            wo = per.tile([128, 2, D], F32R)       # Wo^T for our head cols
            qt = per.tile([128, 2, S], FP16)       # Q^T (2 heads per tile)
            kt = per.tile([128, 2, S], FP16)
            vaug = per.tile([128, 16, 4 * (HD + 1)], FP16)  # V + ones col per head
            ctxn = per.tile([128, 2, S], F32R)     # normalized ctx^T
            tri = per.tile([128, 128], FP16)       # tri[kk,c]=1 iff kk<=c
            wq = per.tile([128, 8, GD], F32R)
            wk = per.tile([128, 8, GD], F32R)
            wv = per.tile([128, 8, GD], F32R)

            make_upper_triangular(nc, tri[:, :], val=1.0, diag=True)

            # DMA waves, matched to consumption order
            def eng(n):
                return nc.sync if n % 2 == 0 else nc.scalar

            for dt in range(8):
                eng(dt).dma_start(wq[:, dt, :], wq_d[128 * dt:128 * dt + 128, :])
                eng(dt + 1).dma_start(xt[:, dt, 0:512], xt_d[128 * dt:128 * dt + 128, 0:512])
            for dt in range(8):
                eng(dt).dma_start(wv[:, dt, :], wv_d[128 * dt:128 * dt + 128, :])
                eng(dt + 1).dma_start(xt[:, dt, 512:1024], xt_d[128 * dt:128 * dt + 128, 512:1024])
            for dt in range(8):
                eng(dt).dma_start(wk[:, dt, :], wk_d[128 * dt:128 * dt + 128, :])
                eng(dt + 1).dma_start(xt[:, dt, 1024:1536], xt_d[128 * dt:128 * dt + 128, 1024:1536])
            for dt in range(8):
                eng(dt).dma_start(xt[:, dt, 1536:2048], xt_d[128 * dt:128 * dt + 128, 1536:2048])
            for dp in range(2):
                nc.sync.dma_start(wo[:, dp, :], wo_d[128 * dp:128 * dp + 128, :])

            def emit_qk(w_t, dst, sc):
                for dp in range(2):
                    ps = ps_a.tile([128, 512], F32, tag="blk")
                    for dt in range(8):
                        nc.tensor.matmul(
                            ps[:, :],
                            w_t[:, dt, 128 * dp:128 * dp + 128],
                            xt[:, dt, 512 * sc:512 * sc + 512],
                            start=(dt == 0), stop=(dt == 7),
                        )
                    nc.vector.tensor_copy(dst[:, dp, 512 * sc:512 * sc + 512], ps[:, :])

            def emit_v(sc):
                for st in range(4 * sc, 4 * sc + 4):
                    psv = ps_a.tile([128, 256], F32, tag="blk")
                    for dt in range(8):
                        nc.tensor.matmul(
                            psv[:, :],
                            xt[:, dt, 128 * st:128 * st + 128],
                            wv[:, dt, :],
                            start=(dt == 0), stop=(dt == 7),
                        )
                    v_dst = vaug[:, st, :].rearrange("p (h c) -> p h c", c=HD + 1)
                    nc.vector.tensor_copy(
                        v_dst[:, :, 0:HD],
                        psv.rearrange("p (h c) -> p h c", c=HD),
                    )
                    # ones column: x*0+1 through DVE so the write is rounded
                    nc.vector.tensor_scalar(
                        v_dst[:, :, HD:HD + 1], psv[:, 0:4], 0.0, 1.0,
                        op0=Alu.mult, op1=Alu.add,
                    )

            def emit_head_pair(qc, i):
                """Heads hA=2i (PE rows 0-63) and hB=2i+1 (rows 64-127): their
                score matmuls are emitted alternating so the hardware runs
                them concurrently in disjoint PE row groups."""
                hA, hB = 2 * i, 2 * i + 1
                heads = ((hA, 0), (hB, 64))
                ctxs = {}
                pts = {h: [] for h, _ in heads}
                packs = [
                    (896, ((0, 0, 512), (1, 512, 384))),
                    (384, ((3, 0, 128), (2, 128, 256))),
                ]
                for h, qo in heads:
                    ctx_t = ps_c.tile([65, 512], F32, tag="ctx")
                    ctxs[h] = ctx_t
                # diagonal strips: A and B tiles in flight together, matmuls
                # alternating between the two row groups
                for width, parts in packs:
                    sps = {}
                    for h, qo in heads:
                        sp_t = ps_a.tile([128, 1024], F32, tag="blk")
                        sps[h] = sp_t
                    for j, o, w in parts:
                        k_t = 4 * qc + j
                        for h, qo in heads:
                            nc.tensor.matmul(
                                sps[h][:, o:o + w],
                                kt[qo:qo + 64, i, 128 * k_t:128 * k_t + 128],
                                qt[qo:qo + 64, i, 512 * qc + 128 * j:512 * qc + 128 * j + w],
                                start=True, stop=True,
                            )
                    for h, qo in heads:
                        pt_p = ptp.tile([128, 1024], FP16, tag="pt")
                        nc.scalar.activation(pt_p[:, :width], sps[h][:, :width], Exp, scale=0.125)
                        for ii, (j, o, w) in enumerate(parts):
                            eng = nc.vector if ii == 0 else nc.gpsimd
                            eng.tensor_mul(
                                pt_p[:, o:o + 128], pt_p[:, o:o + 128], tri[:, :]
                            )
                        pts[h].append((pt_p, parts))
                # full blocks (2 k-tiles per tile), pairwise
                for blk in range(2 * qc):
                    sps = {}
                    for h, qo in heads:
                        sp_t = ps_a.tile([128, 1024], F32, tag="blk")
                        sps[h] = sp_t
                    for j2 in range(2):
                        k_t = 2 * blk + j2
                        for h, qo in heads:
                            nc.tensor.matmul(
                                sps[h][:, 512 * j2:512 * j2 + 512],
                                kt[qo:qo + 64, i, 128 * k_t:128 * k_t + 128],
                                qt[qo:qo + 64, i, 512 * qc:512 * qc + 512],
                                start=True, stop=True,
                            )
                    for h, qo in heads:
                        pt_b = ptp.tile([128, 1024], FP16, tag="pt")
                        nc.scalar.activation(pt_b[:, :], sps[h][:, :], Exp, scale=0.125)
                        pts[h].append((pt_b, ((None, 0, 512), (None, 512, 512))))
                # ctx accumulation per head
                for h, qo in heads:
                    ctx = ctxs[h]
                    ctx_mms = []
                    for bi, (pt_t, parts) in enumerate(pts[h]):
                        for pj, (j, o, w) in enumerate(parts):
                            if bi < 2:          # diagonal strip tiles
                                k_t, co = 4 * qc + j, 128 * j
                            else:               # full block tiles
                                k_t, co = 2 * (bi - 2) + pj, 0
                            ctx_mms.append((pt_t, k_t, o, w, co))
                    for n, (pt_t, k_t, o, w, co) in enumerate(ctx_mms):
                        nc.tensor.matmul(
                            ctx[:, co:co + w],
                            vaug[:, k_t, 65 * h:65 * h + 65],
                            pt_t[:, o:o + w],
                            start=(n == 0), stop=(n == len(ctx_mms) - 1),
                        )
                # normalize both heads: l rows to SBUF, one recip, broadcast,
                # scale each head's PSUM ctx into ctxn
                for n, (h, qo) in enumerate(heads):
                    l_sb = wk1.tile([1, 512], F32, tag="lrow")
                    nc.vector.tensor_copy(l_sb[:, :], ctxs[h][64:65, :])
                    r_sb = wk1.tile([1, 512], F32, tag="rrow")
                    nc.vector.reciprocal_approx_fast(r_sb[:, :], l_sb[:, :])
                    rb = wk1.tile([64, 512], F32, tag="rb")
                    nc.gpsimd.partition_broadcast(rb[:, :], r_sb[:1, :], channels=64)
                    nc.vector.tensor_mul(
                        ctxn[qo:qo + 64, i, 512 * qc:512 * qc + 512],
                        ctxs[h][0:64, :], rb[:, :],
                    )

            def emit_outproj(qc, drain=None):
                # bias is added on the host during the unshard sum; the PSUM
                # drain alternates engines (late chunks use the scalar engine,
                # idle once the exps are done)
                for st in range(4 * qc, 4 * qc + 4):
                    for oc in range(2):
                        dma_eng = nc.sync if (st + oc) % 2 == 0 else nc.scalar
                        pso = ps_o.tile([128, 512], F32, tag="po")
                        for dp in range(2):
                            nc.tensor.matmul(
                                pso[:, :],
                                ctxn[:, dp, 128 * st:128 * st + 128],
                                wo[:, dp, 512 * oc:512 * oc + 512],
                                start=(dp == 0), stop=(dp == 1),
                            )
                        ob = wk2.tile([128, 512], F32, tag="ob")
                        if drain is None:
                            nc.vector.tensor_copy(ob[:, :], pso[:, :])
                        elif (st + oc) % 2 == 0:
                            drain(ob[:, :], pso[:, :])
                        else:
                            nc.vector.tensor_copy(ob[:, :], pso[:, :])
                        dma_eng.dma_start(
                            out_d[128 * st:128 * st + 128, 512 * oc:512 * oc + 512],
                            ob[:, :],
                        )

            # ---- projection waves with chunk-0 attention folded in ----
            emit_qk(wq, qt, 0)
            emit_qk(wq, qt, 1)
            emit_v(0)
            emit_v(1)
            emit_qk(wk, kt, 0)
            emit_head_pair(0, 0)
            emit_head_pair(0, 1)
            emit_qk(wq, qt, 2)
            emit_qk(wk, kt, 1)
            emit_v(2)
            emit_head_pair(1, 0)
            emit_qk(wq, qt, 3)
            emit_qk(wk, kt, 2)
            emit_v(3)
            emit_qk(wk, kt, 3)
            emit_outproj(0)

            # ---- chunks 3/2/1 interleaved; outprojs woven in ----
            emit_head_pair(3, 0)
            emit_head_pair(2, 0)
            emit_head_pair(3, 1)
            emit_head_pair(2, 1)
            emit_outproj(3)
            emit_head_pair(1, 1)
            emit_outproj(2)
            emit_outproj(1, drain=nc.scalar.copy)
    nc.compile()
    return nc


_NC = None


def _get_nc():
    global _NC
    if _NC is None:
        _NC = _build()
    return _NC


def kernel(**inputs):
    x = np.asarray(inputs["inputs"], dtype=np.float32)
    wq = np.asarray(inputs["Wq"], dtype=np.float32)
    wk = np.asarray(inputs["Wk"], dtype=np.float32)
    wv = np.asarray(inputs["Wv"], dtype=np.float32)
    wo = np.asarray(inputs["Wo"], dtype=np.float32)
    bo = np.asarray(inputs["bo"], dtype=np.float32)

    xts = [np.ascontiguousarray(x[b].T) for b in range(B)]
    in_maps = []
    for c in range(N_CORES):
        b, g = c // 4, c % 4
        sl = slice(GD * g, GD * g + GD)
        in_maps.append({
            "xt": xts[b],
            "wqT": np.ascontiguousarray(wq[sl, :].T),
            "wkT": np.ascontiguousarray(wk[sl, :].T),
            "wvT": np.ascontiguousarray(wv[sl, :].T),
            "woT": np.ascontiguousarray(wo[:, sl].T),
        })

    nc = _get_nc()
    res = run_bass_kernel_spmd(nc, in_maps, core_ids=list(range(N_CORES)))
    out = np.empty((B, S, D), np.float32)
    for b in range(B):
        acc = res.results[4 * b + 0]["out"].astype(np.float32)
        for g in range(1, 4):
            acc = acc + res.results[4 * b + g]["out"]
        out[b] = acc + bo
    return out



# revision 56
# speedup vs baseline: 1.0561x; 1.0561x over previous
"""Causal multi-head attention (B=2, S=2048, D=1024, H=16, HD=64) on 8 NeuronCores.

Sharding: core c = 4*b + g handles batch b (2-way data parallel) and head
group g (4-way tensor parallel over the 16 heads, 4 heads per core).
Each core computes its 4 heads' attention plus the partial output
projection (columns of Wo for its heads); the host sums the 4 partials
per batch ("row-parallel" reduction) to produce the full output.

Device layout notes:
  - All device tensors are fp16: matmuls run at the same 1 cycle/row as
    fp32r but DMA bytes halve (5e-4 absmax-relative error in numpy).
  - X is fed transposed (xt = X[b].T, [D,S]) so the d-contraction of the
    QKV projections has d on SBUF partitions.
  - Q,K are produced transposed ([d_head, s]); scores are computed
    transposed (S^T[k,q]) so the P@V matmul needs no transposes at all.
  - V is produced in natural [s, d] layout, augmented with a ones column
    per head so the P@V matmul also yields the softmax denominator.
  - softmax skips max-subtraction (scores/8 ~ N(0,1); exp is safe in f32).
  - Inputs stream in ~17 large DMAs on the SP + Act queues (Pool/gpsimd
    DMAs hang on this stack); output partials are stored fp16, two row
    blocks per DMA.
  - chunks 0 and 1 of the attention run inside the DMA-bound projection
    window (the scalar engine is idle there); the exp-heavy chunks 3 and 2
    run interleaved afterwards.
"""

import numpy as np

import concourse.mybir as mybir
from concourse import bacc
from concourse.tile import TileContext
from concourse.bass_utils import run_bass_kernel_spmd
from concourse.masks import make_upper_triangular

F32 = mybir.dt.float32
F16 = mybir.dt.float16
FP16 = mybir.dt.float16
Exp = mybir.ActivationFunctionType.Exp
Alu = mybir.AluOpType

B, S, D, H, HD = 2, 2048, 1024, 16, 64
GH = 4            # heads per core
GD = GH * HD      # 256 features per core
N_CORES = 8


def _build():
    nc = bacc.Bacc("TRN2", target_bir_lowering=False, name="mha_tp")
    xt_d = nc.dram_tensor("xt", [D, S], F16, kind="ExternalInput")
    wq_d = nc.dram_tensor("wqT", [D, GD], F16, kind="ExternalInput")
    wk_d = nc.dram_tensor("wkT", [D, GD], F16, kind="ExternalInput")
    wv_d = nc.dram_tensor("wvT", [D, GD], F16, kind="ExternalInput")
    wo_d = nc.dram_tensor("woT", [GD, D], F16, kind="ExternalInput")
    out_d = nc.dram_tensor("out", [S, D], F16, kind="ExternalOutput")

    with TileContext(nc) as tc:
        with (
            tc.tile_pool(name="per", bufs=1) as per,
            tc.tile_pool(name="pt", bufs=10) as ptp,
            tc.tile_pool(name="wk1", bufs=2) as wk1,
            tc.tile_pool(name="ob", bufs=6) as obp,
            tc.tile_pool(name="ps_a", bufs=2, space="PSUM") as ps_a,
            tc.tile_pool(name="ps_o", bufs=2, space="PSUM") as ps_o,
            tc.tile_pool(name="ps_c", bufs=2, space="PSUM") as ps_c,
        ):
            xt = per.tile([128, 8, S], F16)        # X^T, d-tile major
            wo = per.tile([128, 2, D], F16)        # Wo^T for our head cols
            qt = per.tile([128, 2, S], FP16)       # Q^T (2 heads per tile)
            kt = per.tile([128, 2, S], FP16)
            vaug = per.tile([128, 16, 4 * (HD + 1)], FP16)  # V + ones col per head
            ctxn = per.tile([128, 2, S], F16)      # normalized ctx^T
            tri = per.tile([128, 128], FP16)       # tri[kk,c]=1 iff kk<=c
            wq = per.tile([128, 8, GD], F16)
            wk = per.tile([128, 8, GD], F16)
            wv = per.tile([128, 8, GD], F16)

            # ---- input DMA: large pieces, SP + Act queues, consumption
            # order: wq, xt0, wv, xt1, wk, xt2, xt3, wo ----
            def ld_x(eng, qlo, qhi, c0, c1):
                eng.dma_start(
                    xt[:, qlo:qhi, c0:c1],
                    xt_d[128 * qlo:128 * qhi, c0:c1].rearrange(
                        "(t p) c -> p t c", p=128),
                )

            def ld_w(eng, w_t, w_d, qlo, qhi):
                eng.dma_start(
                    w_t[:, qlo:qhi, :],
                    w_d[128 * qlo:128 * qhi, :].rearrange(
                        "(t p) c -> p t c", p=128),
                )

            ld_w(nc.sync, wq, wq_d, 0, 4)          # 256KB pieces
            make_upper_triangular(nc, tri[:, :], val=1.0, diag=True)
            ld_x(nc.scalar, 0, 4, 0, 512)
            ld_w(nc.sync, wq, wq_d, 4, 8)
            ld_x(nc.scalar, 4, 8, 0, 512)
            ld_x(nc.sync, 0, 8, 512, 1024)         # xt1 feeds Q1 next
            ld_w(nc.scalar, wv, wv_d, 0, 8)        # 512KB pieces from here
            ld_w(nc.sync, wk, wk_d, 0, 8)
            ld_x(nc.scalar, 0, 8, 1024, 1536)
            ld_x(nc.sync, 0, 8, 1536, 2048)
            nc.scalar.dma_start(
                wo[:, :, :],
                wo_d[:, :].rearrange("(t p) c -> p t c", p=128),
            )

            def emit_qk(w_t, dst, sc):
                for dp in range(2):
                    ps = ps_a.tile([128, 512], F32, tag="blk")
                    for dt in range(8):
                        nc.tensor.matmul(
                            ps[:, :],
                            w_t[:, dt, 128 * dp:128 * dp + 128],
                            xt[:, dt, 512 * sc:512 * sc + 512],
                            start=(dt == 0), stop=(dt == 7),
                        )
                    nc.vector.tensor_copy(dst[:, dp, 512 * sc:512 * sc + 512], ps[:, :])

            def emit_v(sc):
                for st in range(4 * sc, 4 * sc + 4):
                    psv = ps_a.tile([128, 256], F32, tag="blk")
                    for dt in range(8):
                        nc.tensor.matmul(
                            psv[:, :],
                            xt[:, dt, 128 * st:128 * st + 128],
                            wv[:, dt, :],
                            start=(dt == 0), stop=(dt == 7),
                        )
                    v_dst = vaug[:, st, :].rearrange("p (h c) -> p h c", c=HD + 1)
                    nc.vector.tensor_copy(
                        v_dst[:, :, 0:HD],
                        psv.rearrange("p (h c) -> p h c", c=HD),
                    )
                    # ones column: x*0+1 through DVE so the write is rounded
                    nc.vector.tensor_scalar(
                        v_dst[:, :, HD:HD + 1], psv[:, 0:4], 0.0, 1.0,
                        op0=Alu.mult, op1=Alu.add,
                    )

            def emit_head_pair(qc, i):
                """Heads hA=2i (PE rows 0-63) and hB=2i+1 (rows 64-127): their
                score matmuls are emitted alternating so the hardware runs
                them concurrently in disjoint PE row groups."""
                hA, hB = 2 * i, 2 * i + 1
                heads = ((hA, 0), (hB, 64))
                ctxs = {}
                pts = {h: [] for h, _ in heads}
                packs = [
                    (896, ((0, 0, 512), (1, 512, 384))),
                    (384, ((3, 0, 128), (2, 128, 256))),
                ]
                for h, qo in heads:
                    ctx_t = ps_c.tile([65, 512], F32, tag="ctx")
                    ctxs[h] = ctx_t
                # diagonal strips: A and B tiles in flight together, matmuls
                # alternating between the two row groups
                for width, parts in packs:
                    sps = {}
                    for h, qo in heads:
                        sp_t = ps_a.tile([128, 1024], F32, tag="blk")
                        sps[h] = sp_t
                    for j, o, w in parts:
                        k_t = 4 * qc + j
                        for h, qo in heads:
                            nc.tensor.matmul(
                                sps[h][:, o:o + w],
                                kt[qo:qo + 64, i, 128 * k_t:128 * k_t + 128],
                                qt[qo:qo + 64, i, 512 * qc + 128 * j:512 * qc + 128 * j + w],
                                start=True, stop=True,
                            )
                    for h, qo in heads:
                        pt_p = ptp.tile([128, 1024], FP16, tag="pt")
                        nc.scalar.activation(pt_p[:, :width], sps[h][:, :width], Exp, scale=0.125)
                        for ii, (j, o, w) in enumerate(parts):
                            engm = nc.vector if ii == 0 else nc.gpsimd
                            engm.tensor_mul(
                                pt_p[:, o:o + 128], pt_p[:, o:o + 128], tri[:, :]
                            )
                        pts[h].append((pt_p, parts))
                # full blocks (2 k-tiles per tile), pairwise
                for blk in range(2 * qc):
                    sps = {}
                    for h, qo in heads:
                        sp_t = ps_a.tile([128, 1024], F32, tag="blk")
                        sps[h] = sp_t
                    for j2 in range(2):
                        k_t = 2 * blk + j2
                        for h, qo in heads:
                            nc.tensor.matmul(
                                sps[h][:, 512 * j2:512 * j2 + 512],
                                kt[qo:qo + 64, i, 128 * k_t:128 * k_t + 128],
                                qt[qo:qo + 64, i, 512 * qc:512 * qc + 512],
                                start=True, stop=True,
                            )
                    for h, qo in heads:
                        pt_b = ptp.tile([128, 1024], FP16, tag="pt")
                        nc.scalar.activation(pt_b[:, :], sps[h][:, :], Exp, scale=0.125)
                        pts[h].append((pt_b, ((None, 0, 512), (None, 512, 512))))
                # ctx accumulation per head
                for h, qo in heads:
                    ctx = ctxs[h]
                    ctx_mms = []
                    for bi, (pt_t, parts) in enumerate(pts[h]):
                        for pj, (j, o, w) in enumerate(parts):
                            if bi < 2:          # diagonal strip tiles
                                k_t, co = 4 * qc + j, 128 * j
                            else:               # full block tiles
                                k_t, co = 2 * (bi - 2) + pj, 0
                            ctx_mms.append((pt_t, k_t, o, w, co))
                    for n, (pt_t, k_t, o, w, co) in enumerate(ctx_mms):
                        nc.tensor.matmul(
                            ctx[:, co:co + w],
                            vaug[:, k_t, 65 * h:65 * h + 65],
                            pt_t[:, o:o + w],
                            start=(n == 0), stop=(n == len(ctx_mms) - 1),
                        )
                # normalize both heads: l row to SBUF, recip, broadcast,
                # scale each head's PSUM ctx into ctxn
                for n, (h, qo) in enumerate(heads):
                    l_sb = wk1.tile([1, 512], F32, tag="lrow")
                    nc.vector.tensor_copy(l_sb[:, :], ctxs[h][64:65, :])
                    r_sb = wk1.tile([1, 512], F32, tag="rrow")
                    nc.vector.reciprocal_approx_fast(r_sb[:, :], l_sb[:, :])
                    rb = wk1.tile([64, 512], F32, tag="rb")
                    nc.gpsimd.partition_broadcast(rb[:, :], r_sb[:1, :], channels=64)
                    nc.vector.tensor_mul(
                        ctxn[qo:qo + 64, i, 512 * qc:512 * qc + 512],
                        ctxs[h][0:64, :], rb[:, :],
                    )

            def emit_outproj(qc, drain=None):
                # bias is added on the host during the unshard sum; output
                # rows accumulate into fp16 SBUF tiles and ship two row
                # blocks per DMA (HWDGE path)
                for sp in range(2):
                    st0 = 4 * qc + 2 * sp
                    ob = obp.tile([128, 2, D], F16, tag="ob", name="ob")
                    for sti in range(2):
                        st = st0 + sti
                        for oc in range(2):
                            pso = ps_o.tile([128, 512], F32, tag="po")
                            for dp in range(2):
                                nc.tensor.matmul(
                                    pso[:, :],
                                    ctxn[:, dp, 128 * st:128 * st + 128],
                                    wo[:, dp, 512 * oc:512 * oc + 512],
                                    start=(dp == 0), stop=(dp == 1),
                                )
                            dst = ob[:, sti, 512 * oc:512 * oc + 512]
                            if drain is not None and (st + oc) % 2 == 0:
                                drain(dst, pso[:, :])
                            else:
                                nc.vector.tensor_copy(dst, pso[:, :])
                    nc.sync.dma_start(
                        out_d[128 * st0:128 * st0 + 256, :].rearrange(
                            "(t p) c -> p t c", p=128),
                        ob[:, :, :],
                    )

            # ---- projection waves with chunk-0 attention folded in ----
            emit_qk(wq, qt, 0)
            emit_qk(wq, qt, 1)
            emit_v(0)
            emit_v(1)
            emit_qk(wk, kt, 0)
            emit_head_pair(0, 0)
            emit_head_pair(0, 1)
            emit_qk(wq, qt, 2)
            emit_qk(wk, kt, 1)
            emit_v(2)
            emit_head_pair(1, 0)
            emit_qk(wq, qt, 3)
            emit_qk(wk, kt, 2)
            emit_v(3)
            emit_qk(wk, kt, 3)
            emit_outproj(0, drain=nc.scalar.copy)

            # ---- chunks 3/2/1 interleaved; outprojs woven in ----
            emit_head_pair(3, 0)
            emit_head_pair(2, 0)
            emit_head_pair(3, 1)
            emit_head_pair(2, 1)
            emit_outproj(3)
            emit_head_pair(1, 1)
            emit_outproj(2)
            emit_outproj(1, drain=nc.scalar.copy)
    nc.compile()
    return nc


_NC = None


def _get_nc():
    global _NC
    if _NC is None:
        _NC = _build()
    return _NC


def kernel(**inputs):
    x = np.asarray(inputs["inputs"], dtype=np.float32)
    wq = np.asarray(inputs["Wq"], dtype=np.float32)
    wk = np.asarray(inputs["Wk"], dtype=np.float32)
    wv = np.asarray(inputs["Wv"], dtype=np.float32)
    wo = np.asarray(inputs["Wo"], dtype=np.float32)
    bo = np.asarray(inputs["bo"], dtype=np.float32)

    xts = [np.ascontiguousarray(x[b].T).astype(np.float16) for b in range(B)]
    in_maps = []
    for c in range(N_CORES):
        b, g = c // 4, c % 4
        sl = slice(GD * g, GD * g + GD)
        in_maps.append({
            "xt": xts[b],
            "wqT": np.ascontiguousarray(wq[sl, :].T).astype(np.float16),
            "wkT": np.ascontiguousarray(wk[sl, :].T).astype(np.float16),
            "wvT": np.ascontiguousarray(wv[sl, :].T).astype(np.float16),
            "woT": np.ascontiguousarray(wo[:, sl].T).astype(np.float16),
        })

    nc = _get_nc()
    res = run_bass_kernel_spmd(nc, in_maps, core_ids=list(range(N_CORES)))
    out = np.empty((B, S, D), np.float32)
    for b in range(B):
        acc = res.results[4 * b + 0]["out"].astype(np.float32)
        for g in range(1, 4):
            acc = acc + res.results[4 * b + g]["out"].astype(np.float32)
        out[b] = acc + bo
    return out


# revision 65
# speedup vs baseline: 1.0828x; 1.0253x over previous
"""Causal multi-head attention (B=2, S=2048, D=1024, H=16, HD=64) on 8 NeuronCores.

Sharding: core c = 4*b + g handles batch b (2-way data parallel) and head
group g (4-way tensor parallel over the 16 heads, 4 heads per core).
Each core computes its 4 heads' attention plus the partial output
projection (columns of Wo for its heads); the host sums the 4 partials
per batch ("row-parallel" reduction) to produce the full output.

Device layout notes:
  - All device tensors are fp16: matmuls run at the same 1 cycle/row as
    fp32r but DMA bytes halve (5e-4 absmax-relative error in numpy).
  - X is fed transposed (xt = X[b].T, [D,S]) so the d-contraction of the
    QKV projections has d on SBUF partitions.
  - Q,K are produced transposed ([d_head, s]); scores are computed
    transposed (S^T[k,q]) so the P@V matmul needs no transposes at all.
  - V is produced in natural [s, d] layout, augmented with a ones column
    per head so the P@V matmul also yields the softmax denominator.
  - softmax skips max-subtraction (scores/8 ~ N(0,1); exp is safe in f32).
  - Inputs stream in ~17 large DMAs on the SP + Act queues (Pool/gpsimd
    DMAs hang on this stack); output partials are stored fp16, two row
    blocks per DMA.
  - chunks 0 and 1 of the attention run inside the DMA-bound projection
    window (the scalar engine is idle there); the exp-heavy chunks 3 and 2
    run interleaved afterwards.
"""

import numpy as np

import concourse.mybir as mybir
from concourse import bacc
from concourse.tile import TileContext
from concourse.bass_utils import run_bass_kernel_spmd
from concourse.masks import make_upper_triangular

F32 = mybir.dt.float32
F16 = mybir.dt.float16
FP16 = mybir.dt.float16
Exp = mybir.ActivationFunctionType.Exp
Alu = mybir.AluOpType

B, S, D, H, HD = 2, 2048, 1024, 16, 64
GH = 4            # heads per core
GD = GH * HD      # 256 features per core
N_CORES = 8


def _build():
    nc = bacc.Bacc("TRN2", target_bir_lowering=False, name="mha_tp")
    xt_d = nc.dram_tensor("xt", [D, S], F16, kind="ExternalInput")
    wq_d = nc.dram_tensor("wqT", [D, GD], F16, kind="ExternalInput")
    wk_d = nc.dram_tensor("wkT", [D, GD], F16, kind="ExternalInput")
    wv_d = nc.dram_tensor("wvT", [D, GD], F16, kind="ExternalInput")
    wo_d = nc.dram_tensor("woT", [GD, D], F16, kind="ExternalInput")
    out_d = nc.dram_tensor("out", [S, D], F16, kind="ExternalOutput")

    with TileContext(nc) as tc:
        with (
            tc.tile_pool(name="per", bufs=1) as per,
            tc.tile_pool(name="pt", bufs=10) as ptp,
            tc.tile_pool(name="wk1", bufs=2) as wk1,
            tc.tile_pool(name="ob", bufs=6) as obp,
            tc.tile_pool(name="ps_a", bufs=2, space="PSUM") as ps_a,
            tc.tile_pool(name="ps_o", bufs=2, space="PSUM") as ps_o,
            tc.tile_pool(name="ps_c", bufs=2, space="PSUM") as ps_c,
        ):
            xt = per.tile([128, 8, S], F16)        # X^T, d-tile major
            wo = per.tile([128, 2, D], F16)        # Wo^T for our head cols
            qt = per.tile([128, 2, S], FP16)       # Q^T (2 heads per tile)
            kt = per.tile([128, 2, S], FP16)
            vaug = per.tile([128, 16, 4 * (HD + 1)], FP16)  # V + ones col per head
            ctxn = per.tile([128, 2, S], F16)      # normalized ctx^T
            tri = per.tile([128, 128], FP16)       # tri[kk,c]=1 iff kk<=c
            wq = per.tile([128, 8, GD], F16)
            wk = per.tile([128, 8, GD], F16)
            wv = per.tile([128, 8, GD], F16)

            # ---- input DMA: large pieces, SP + Act queues, consumption
            # order: wq, xt0, wv, xt1, wk, xt2, xt3, wo ----
            def ld_x(eng, qlo, qhi, c0, c1):
                eng.dma_start(
                    xt[:, qlo:qhi, c0:c1],
                    xt_d[128 * qlo:128 * qhi, c0:c1].rearrange(
                        "(t p) c -> p t c", p=128),
                )

            def ld_w(eng, w_t, w_d, qlo, qhi):
                eng.dma_start(
                    w_t[:, qlo:qhi, :],
                    w_d[128 * qlo:128 * qhi, :].rearrange(
                        "(t p) c -> p t c", p=128),
                )

            ld_w(nc.sync, wq, wq_d, 0, 2)          # small first pieces
            make_upper_triangular(nc, tri[:, :], val=1.0, diag=True)
            ld_x(nc.scalar, 0, 2, 0, 512)
            ld_w(nc.sync, wq, wq_d, 2, 8)
            ld_x(nc.scalar, 2, 8, 0, 512)
            ld_w(nc.sync, wk, wk_d, 0, 8)          # K0 right after Q0
            ld_w(nc.scalar, wv, wv_d, 0, 8)
            ld_x(nc.sync, 0, 8, 512, 1024)
            ld_x(nc.scalar, 0, 8, 1024, 1536)
            ld_x(nc.sync, 0, 8, 1536, 2048)
            nc.scalar.dma_start(
                wo[:, :, :],
                wo_d[:, :].rearrange("(t p) c -> p t c", p=128),
            )

            def emit_qk(w_t, dst, sc):
                for dp in range(2):
                    ps = ps_a.tile([128, 512], F32, tag="blk")
                    for dt in range(8):
                        nc.tensor.matmul(
                            ps[:, :],
                            w_t[:, dt, 128 * dp:128 * dp + 128],
                            xt[:, dt, 512 * sc:512 * sc + 512],
                            start=(dt == 0), stop=(dt == 7),
                        )
                    nc.vector.tensor_copy(dst[:, dp, 512 * sc:512 * sc + 512], ps[:, :])

            def emit_v(sc):
                for st in range(4 * sc, 4 * sc + 4):
                    psv = ps_a.tile([128, 256], F32, tag="blk")
                    for dt in range(8):
                        nc.tensor.matmul(
                            psv[:, :],
                            xt[:, dt, 128 * st:128 * st + 128],
                            wv[:, dt, :],
                            start=(dt == 0), stop=(dt == 7),
                        )
                    v_dst = vaug[:, st, :].rearrange("p (h c) -> p h c", c=HD + 1)
                    nc.vector.tensor_copy(
                        v_dst[:, :, 0:HD],
                        psv.rearrange("p (h c) -> p h c", c=HD),
                    )
                    # ones column: x*0+1 through DVE so the write is rounded
                    nc.vector.tensor_scalar(
                        v_dst[:, :, HD:HD + 1], psv[:, 0:4], 0.0, 1.0,
                        op0=Alu.mult, op1=Alu.add,
                    )

            def emit_head_pair(qc, i, defer_norm=False):
                """Heads hA=2i (PE rows 0-63) and hB=2i+1 (rows 64-127): their
                score matmuls are emitted alternating so the hardware runs
                them concurrently in disjoint PE row groups."""
                hA, hB = 2 * i, 2 * i + 1
                heads = ((hA, 0), (hB, 64))
                ctxs = {}
                pts = {h: [] for h, _ in heads}
                packs = [
                    (896, ((0, 0, 512), (1, 512, 384))),
                    (384, ((3, 0, 128), (2, 128, 256))),
                ]
                for h, qo in heads:
                    ctx_t = ps_c.tile([65, 512], F32, tag="ctx")
                    ctxs[h] = ctx_t
                # diagonal strips: A and B tiles in flight together, matmuls
                # alternating between the two row groups
                for width, parts in packs:
                    sps = {}
                    for h, qo in heads:
                        sp_t = ps_a.tile([128, 1024], F32, tag="blk")
                        sps[h] = sp_t
                    for j, o, w in parts:
                        k_t = 4 * qc + j
                        for h, qo in heads:
                            nc.tensor.matmul(
                                sps[h][:, o:o + w],
                                kt[qo:qo + 64, i, 128 * k_t:128 * k_t + 128],
                                qt[qo:qo + 64, i, 512 * qc + 128 * j:512 * qc + 128 * j + w],
                                start=True, stop=True,
                            )
                    for h, qo in heads:
                        pt_p = ptp.tile([128, 1024], FP16, tag="pt")
                        nc.scalar.activation(pt_p[:, :width], sps[h][:, :width], Exp, scale=0.125)
                        for ii, (j, o, w) in enumerate(parts):
                            engm = nc.vector if ii == 0 else nc.gpsimd
                            engm.tensor_mul(
                                pt_p[:, o:o + 128], pt_p[:, o:o + 128], tri[:, :]
                            )
                        pts[h].append((pt_p, parts))
                # full blocks (2 k-tiles per tile), pairwise
                for blk in range(2 * qc):
                    sps = {}
                    for h, qo in heads:
                        sp_t = ps_a.tile([128, 1024], F32, tag="blk")
                        sps[h] = sp_t
                    for j2 in range(2):
                        k_t = 2 * blk + j2
                        for h, qo in heads:
                            nc.tensor.matmul(
                                sps[h][:, 512 * j2:512 * j2 + 512],
                                kt[qo:qo + 64, i, 128 * k_t:128 * k_t + 128],
                                qt[qo:qo + 64, i, 512 * qc:512 * qc + 512],
                                start=True, stop=True,
                            )
                    for h, qo in heads:
                        pt_b = ptp.tile([128, 1024], FP16, tag="pt")
                        nc.scalar.activation(pt_b[:, :], sps[h][:, :], Exp, scale=0.125)
                        pts[h].append((pt_b, ((None, 0, 512), (None, 512, 512))))
                # ctx accumulation per head
                for h, qo in heads:
                    ctx = ctxs[h]
                    ctx_mms = []
                    for bi, (pt_t, parts) in enumerate(pts[h]):
                        for pj, (j, o, w) in enumerate(parts):
                            if bi < 2:          # diagonal strip tiles
                                k_t, co = 4 * qc + j, 128 * j
                            else:               # full block tiles
                                k_t, co = 2 * (bi - 2) + pj, 0
                            ctx_mms.append((pt_t, k_t, o, w, co))
                    for n, (pt_t, k_t, o, w, co) in enumerate(ctx_mms):
                        nc.tensor.matmul(
                            ctx[:, co:co + w],
                            vaug[:, k_t, 65 * h:65 * h + 65],
                            pt_t[:, o:o + w],
                            start=(n == 0), stop=(n == len(ctx_mms) - 1),
                        )
                if defer_norm:
                    return (qc, i, heads, ctxs)
                emit_normalize((qc, i, heads, ctxs))
                return None

            def emit_normalize(saved):
                # normalize both heads: l row to SBUF, recip, broadcast,
                # scale each head's PSUM ctx into ctxn
                qc, i, heads, ctxs = saved
                for n, (h, qo) in enumerate(heads):
                    l_sb = wk1.tile([1, 512], F32, tag="lrow")
                    nc.vector.tensor_copy(l_sb[:, :], ctxs[h][64:65, :])
                    r_sb = wk1.tile([1, 512], F32, tag="rrow")
                    nc.vector.reciprocal_approx_fast(r_sb[:, :], l_sb[:, :])
                    rb = wk1.tile([64, 512], F32, tag="rb")
                    nc.gpsimd.partition_broadcast(rb[:, :], r_sb[:1, :], channels=64)
                    nc.vector.tensor_mul(
                        ctxn[qo:qo + 64, i, 512 * qc:512 * qc + 512],
                        ctxs[h][0:64, :], rb[:, :],
                    )

            def emit_outproj(qc, drain=None):
                # bias is added on the host during the unshard sum; output
                # rows accumulate into fp16 SBUF tiles and ship two row
                # blocks per DMA (HWDGE path)
                for sp in range(2):
                    st0 = 4 * qc + 2 * sp
                    ob = obp.tile([128, 2, D], F16, tag="ob", name="ob")
                    for sti in range(2):
                        st = st0 + sti
                        for oc in range(2):
                            pso = ps_o.tile([128, 512], F32, tag="po")
                            for dp in range(2):
                                nc.tensor.matmul(
                                    pso[:, :],
                                    ctxn[:, dp, 128 * st:128 * st + 128],
                                    wo[:, dp, 512 * oc:512 * oc + 512],
                                    start=(dp == 0), stop=(dp == 1),
                                )
                            dst = ob[:, sti, 512 * oc:512 * oc + 512]
                            if drain is not None and (st + oc) % 2 == 0:
                                drain(dst, pso[:, :])
                            else:
                                nc.vector.tensor_copy(dst, pso[:, :])
                    nc.sync.dma_start(
                        out_d[128 * st0:128 * st0 + 256, :].rearrange(
                            "(t p) c -> p t c", p=128),
                        ob[:, :, :],
                    )

            # ---- projection waves with chunk-0/1 attention folded in ----
            emit_qk(wq, qt, 0)
            emit_qk(wk, kt, 0)
            emit_v(0)
            emit_head_pair(0, 0)
            emit_head_pair(0, 1)
            emit_qk(wq, qt, 1)
            emit_qk(wk, kt, 1)
            emit_v(1)
            emit_head_pair(1, 0)
            emit_qk(wq, qt, 2)
            emit_qk(wk, kt, 2)
            emit_v(2)
            emit_qk(wq, qt, 3)
            emit_qk(wk, kt, 3)
            emit_v(3)
            emit_outproj(0, drain=nc.scalar.copy)

            # ---- chunks 3/2/1 interleaved; each outproj is emitted right
            # after the pair it truly depends on (cross-engine waits
            # coalesce to the latest emitted DVE work, so emitting an op
            # after an unrelated pair's normalize stalls it falsely) ----
            emit_head_pair(3, 0)
            emit_head_pair(2, 0)
            emit_head_pair(3, 1)
            s21 = emit_head_pair(2, 1, defer_norm=True)
            emit_outproj(3)          # needs only chunk-3 pairs: runs free
            emit_normalize(s21)
            s11 = emit_head_pair(1, 1, defer_norm=True)
            emit_outproj(2, drain=nc.scalar.copy)   # needs only chunk 2
            emit_normalize(s11)
            emit_outproj(1, drain=nc.scalar.copy)
    nc.compile()
    return nc


_NC = None


def _get_nc():
    global _NC
    if _NC is None:
        _NC = _build()
    return _NC


def kernel(**inputs):
    x = np.asarray(inputs["inputs"], dtype=np.float32)
    wq = np.asarray(inputs["Wq"], dtype=np.float32)
    wk = np.asarray(inputs["Wk"], dtype=np.float32)
    wv = np.asarray(inputs["Wv"], dtype=np.float32)
    wo = np.asarray(inputs["Wo"], dtype=np.float32)
    bo = np.asarray(inputs["bo"], dtype=np.float32)

    xts = [np.ascontiguousarray(x[b].T).astype(np.float16) for b in range(B)]
    in_maps = []
    for c in range(N_CORES):
        b, g = c // 4, c % 4
        sl = slice(GD * g, GD * g + GD)
        in_maps.append({
            "xt": xts[b],
            "wqT": np.ascontiguousarray(wq[sl, :].T).astype(np.float16),
            "wkT": np.ascontiguousarray(wk[sl, :].T).astype(np.float16),
            "wvT": np.ascontiguousarray(wv[sl, :].T).astype(np.float16),
            "woT": np.ascontiguousarray(wo[:, sl].T).astype(np.float16),
        })

    nc = _get_nc()
    res = run_bass_kernel_spmd(nc, in_maps, core_ids=list(range(N_CORES)))
    out = np.empty((B, S, D), np.float32)
    for b in range(B):
        acc = res.results[4 * b + 0]["out"].astype(np.float32)
        for g in range(1, 4):
            acc = acc + res.results[4 * b + g]["out"].astype(np.float32)
        out[b] = acc + bo
    return out


# revision 67
# speedup vs baseline: 1.0864x; 1.0033x over previous
"""Causal multi-head attention (B=2, S=2048, D=1024, H=16, HD=64) on 8 NeuronCores.

Sharding: core c = 4*b + g handles batch b (2-way data parallel) and head
group g (4-way tensor parallel over the 16 heads, 4 heads per core).
Each core computes its 4 heads' attention plus the partial output
projection (columns of Wo for its heads); the host sums the 4 partials
per batch ("row-parallel" reduction) to produce the full output.

Device layout notes:
  - All device tensors are fp16: matmuls run at the same 1 cycle/row as
    fp32r but DMA bytes halve (5e-4 absmax-relative error in numpy).
  - X is fed transposed (xt = X[b].T, [D,S]) so the d-contraction of the
    QKV projections has d on SBUF partitions.
  - Q,K are produced transposed ([d_head, s]); scores are computed
    transposed (S^T[k,q]) so the P@V matmul needs no transposes at all.
  - V is produced in natural [s, d] layout, augmented with a ones column
    per head so the P@V matmul also yields the softmax denominator.
  - softmax skips max-subtraction (scores/8 ~ N(0,1); exp is safe in f32).
  - Inputs stream in ~17 large DMAs on the SP + Act queues (Pool/gpsimd
    DMAs hang on this stack); output partials are stored fp16, two row
    blocks per DMA.
  - chunks 0 and 1 of the attention run inside the DMA-bound projection
    window (the scalar engine is idle there); the exp-heavy chunks 3 and 2
    run interleaved afterwards.
"""

import numpy as np

import concourse.mybir as mybir
from concourse import bacc
from concourse.tile import TileContext
from concourse.bass_utils import run_bass_kernel_spmd
from concourse.masks import make_upper_triangular

F32 = mybir.dt.float32
F16 = mybir.dt.float16
FP16 = mybir.dt.float16
Exp = mybir.ActivationFunctionType.Exp
Alu = mybir.AluOpType

B, S, D, H, HD = 2, 2048, 1024, 16, 64
GH = 4            # heads per core
GD = GH * HD      # 256 features per core
N_CORES = 8


def _build():
    nc = bacc.Bacc("TRN2", target_bir_lowering=False, name="mha_tp")
    xt_d = nc.dram_tensor("xt", [D, S], F16, kind="ExternalInput")
    wq_d = nc.dram_tensor("wqT", [D, GD], F16, kind="ExternalInput")
    wk_d = nc.dram_tensor("wkT", [D, GD], F16, kind="ExternalInput")
    wv_d = nc.dram_tensor("wvT", [D, GD], F16, kind="ExternalInput")
    wo_d = nc.dram_tensor("woT", [GD, D], F16, kind="ExternalInput")
    out_d = nc.dram_tensor("out", [S, D], F16, kind="ExternalOutput")

    with TileContext(nc) as tc:
        with (
            tc.tile_pool(name="per", bufs=1) as per,
            tc.tile_pool(name="pt", bufs=10) as ptp,
            tc.tile_pool(name="wk1", bufs=2) as wk1,
            tc.tile_pool(name="ob", bufs=6) as obp,
            tc.tile_pool(name="ps_a", bufs=2, space="PSUM") as ps_a,
            tc.tile_pool(name="ps_o", bufs=2, space="PSUM") as ps_o,
            tc.tile_pool(name="ps_c", bufs=2, space="PSUM") as ps_c,
        ):
            xt = per.tile([128, 8, S], F16)        # X^T, d-tile major
            wo = per.tile([128, 2, D], F16)        # Wo^T for our head cols
            qt = per.tile([128, 2, S], FP16)       # Q^T (2 heads per tile)
            kt = per.tile([128, 2, S], FP16)
            vaug = per.tile([128, 16, 4 * (HD + 1)], FP16)  # V + ones col per head
            ctxn = per.tile([128, 2, S], F16)      # normalized ctx^T
            tri = per.tile([128, 128], FP16)       # tri[kk,c]=1 iff kk<=c
            wq = per.tile([128, 8, GD], F16)
            wk = per.tile([128, 8, GD], F16)
            wv = per.tile([128, 8, GD], F16)

            # ---- input DMA: large pieces, SP + Act queues, consumption
            # order: wq, xt0, wv, xt1, wk, xt2, xt3, wo ----
            def ld_x(eng, qlo, qhi, c0, c1):
                eng.dma_start(
                    xt[:, qlo:qhi, c0:c1],
                    xt_d[128 * qlo:128 * qhi, c0:c1].rearrange(
                        "(t p) c -> p t c", p=128),
                )

            def ld_w(eng, w_t, w_d, qlo, qhi):
                eng.dma_start(
                    w_t[:, qlo:qhi, :],
                    w_d[128 * qlo:128 * qhi, :].rearrange(
                        "(t p) c -> p t c", p=128),
                )

            ld_w(nc.sync, wq, wq_d, 0, 2)          # small first pieces
            make_upper_triangular(nc, tri[:, :], val=1.0, diag=True)
            ld_x(nc.scalar, 0, 2, 0, 512)
            ld_w(nc.sync, wq, wq_d, 2, 8)
            ld_x(nc.scalar, 2, 8, 0, 512)
            ld_w(nc.sync, wk, wk_d, 0, 8)          # K0 right after Q0
            ld_w(nc.scalar, wv, wv_d, 0, 8)
            ld_x(nc.sync, 0, 8, 512, 1024)
            ld_x(nc.scalar, 0, 8, 1024, 1536)
            ld_x(nc.sync, 0, 8, 1536, 2048)
            nc.scalar.dma_start(
                wo[:, :, :],
                wo_d[:, :].rearrange("(t p) c -> p t c", p=128),
            )

            def emit_qk(w_t, dst, sc):
                for dp in range(2):
                    ps = ps_a.tile([128, 512], F32, tag="blk")
                    for dt in range(8):
                        nc.tensor.matmul(
                            ps[:, :],
                            w_t[:, dt, 128 * dp:128 * dp + 128],
                            xt[:, dt, 512 * sc:512 * sc + 512],
                            start=(dt == 0), stop=(dt == 7),
                        )
                    nc.vector.tensor_copy(dst[:, dp, 512 * sc:512 * sc + 512], ps[:, :])

            def emit_v(sc):
                for st in range(4 * sc, 4 * sc + 4):
                    psv = ps_a.tile([128, 256], F32, tag="blk")
                    for dt in range(8):
                        nc.tensor.matmul(
                            psv[:, :],
                            xt[:, dt, 128 * st:128 * st + 128],
                            wv[:, dt, :],
                            start=(dt == 0), stop=(dt == 7),
                        )
                    v_dst = vaug[:, st, :].rearrange("p (h c) -> p h c", c=HD + 1)
                    nc.vector.tensor_copy(
                        v_dst[:, :, 0:HD],
                        psv.rearrange("p (h c) -> p h c", c=HD),
                    )
                    # ones column: x*0+1 through DVE so the write is rounded
                    nc.vector.tensor_scalar(
                        v_dst[:, :, HD:HD + 1], psv[:, 0:4], 0.0, 1.0,
                        op0=Alu.mult, op1=Alu.add,
                    )

            def emit_head_pair(qc, i, defer_norm=False):
                """Heads hA=2i (PE rows 0-63) and hB=2i+1 (rows 64-127): their
                score matmuls are emitted alternating so the hardware runs
                them concurrently in disjoint PE row groups."""
                hA, hB = 2 * i, 2 * i + 1
                heads = ((hA, 0), (hB, 64))
                ctxs = {}
                pts = {h: [] for h, _ in heads}
                packs = [
                    (896, ((0, 0, 512), (1, 512, 384))),
                    (384, ((3, 0, 128), (2, 128, 256))),
                ]
                for h, qo in heads:
                    ctx_t = ps_c.tile([65, 512], F32, tag="ctx")
                    ctxs[h] = ctx_t
                # diagonal strips: A and B tiles in flight together, matmuls
                # alternating between the two row groups
                for width, parts in packs:
                    sps = {}
                    for h, qo in heads:
                        sp_t = ps_a.tile([128, 1024], F32, tag="blk")
                        sps[h] = sp_t
                    for j, o, w in parts:
                        k_t = 4 * qc + j
                        for h, qo in heads:
                            nc.tensor.matmul(
                                sps[h][:, o:o + w],
                                kt[qo:qo + 64, i, 128 * k_t:128 * k_t + 128],
                                qt[qo:qo + 64, i, 512 * qc + 128 * j:512 * qc + 128 * j + w],
                                start=True, stop=True,
                            )
                    for h, qo in heads:
                        pt_p = ptp.tile([128, 1024], FP16, tag="pt")
                        nc.scalar.activation(pt_p[:, :width], sps[h][:, :width], Exp, scale=0.125)
                        for ii, (j, o, w) in enumerate(parts):
                            engm = nc.vector if ii == 0 else nc.gpsimd
                            engm.tensor_mul(
                                pt_p[:, o:o + 128], pt_p[:, o:o + 128], tri[:, :]
                            )
                        pts[h].append((pt_p, parts))
                # full blocks (2 k-tiles per tile), pairwise
                for blk in range(2 * qc):
                    sps = {}
                    for h, qo in heads:
                        sp_t = ps_a.tile([128, 1024], F32, tag="blk")
                        sps[h] = sp_t
                    for j2 in range(2):
                        k_t = 2 * blk + j2
                        for h, qo in heads:
                            nc.tensor.matmul(
                                sps[h][:, 512 * j2:512 * j2 + 512],
                                kt[qo:qo + 64, i, 128 * k_t:128 * k_t + 128],
                                qt[qo:qo + 64, i, 512 * qc:512 * qc + 512],
                                start=True, stop=True,
                            )
                    for h, qo in heads:
                        pt_b = ptp.tile([128, 1024], FP16, tag="pt")
                        nc.scalar.activation(pt_b[:, :], sps[h][:, :], Exp, scale=0.125)
                        pts[h].append((pt_b, ((None, 0, 512), (None, 512, 512))))
                # ctx accumulation per head
                for h, qo in heads:
                    ctx = ctxs[h]
                    ctx_mms = []
                    for bi, (pt_t, parts) in enumerate(pts[h]):
                        for pj, (j, o, w) in enumerate(parts):
                            if bi < 2:          # diagonal strip tiles
                                k_t, co = 4 * qc + j, 128 * j
                            else:               # full block tiles
                                k_t, co = 2 * (bi - 2) + pj, 0
                            ctx_mms.append((pt_t, k_t, o, w, co))
                    for n, (pt_t, k_t, o, w, co) in enumerate(ctx_mms):
                        nc.tensor.matmul(
                            ctx[:, co:co + w],
                            vaug[:, k_t, 65 * h:65 * h + 65],
                            pt_t[:, o:o + w],
                            start=(n == 0), stop=(n == len(ctx_mms) - 1),
                        )
                if defer_norm:
                    return (qc, i, heads, ctxs)
                emit_normalize((qc, i, heads, ctxs))
                return None

            def emit_normalize(saved):
                # normalize both heads: l row to SBUF, recip, broadcast,
                # scale each head's PSUM ctx into ctxn
                qc, i, heads, ctxs = saved
                for n, (h, qo) in enumerate(heads):
                    l_sb = wk1.tile([1, 512], F32, tag="lrow")
                    nc.vector.tensor_copy(l_sb[:, :], ctxs[h][64:65, :])
                    r_sb = wk1.tile([1, 512], F32, tag="rrow")
                    nc.vector.reciprocal_approx_fast(r_sb[:, :], l_sb[:, :])
                    rb = wk1.tile([64, 512], F32, tag="rb")
                    nc.gpsimd.partition_broadcast(rb[:, :], r_sb[:1, :], channels=64)
                    nc.vector.tensor_mul(
                        ctxn[qo:qo + 64, i, 512 * qc:512 * qc + 512],
                        ctxs[h][0:64, :], rb[:, :],
                    )

            def emit_outproj(qc, drain=None, last=False):
                # bias is added on the host during the unshard sum; output
                # rows accumulate into fp16 SBUF tiles and ship two row
                # blocks per DMA (HWDGE path). The LAST chunk stores per-st
                # (and the final st per-half) on parallel queues so the
                # end-of-kernel drain+store chain is as short as possible.
                for sp in range(2):
                    st0 = 4 * qc + 2 * sp
                    if not last:
                        ob = obp.tile([128, 2, D], F16, tag="ob", name="ob")
                    for sti in range(2):
                        st = st0 + sti
                        final = last and sp == 1 and sti == 1
                        if last:
                            ob1 = obp.tile([128, D], F16, tag="ob1", name="ob1")
                        for oc in range(2):
                            pso = ps_o.tile([128, 512], F32, tag="po")
                            for dp in range(2):
                                nc.tensor.matmul(
                                    pso[:, :],
                                    ctxn[:, dp, 128 * st:128 * st + 128],
                                    wo[:, dp, 512 * oc:512 * oc + 512],
                                    start=(dp == 0), stop=(dp == 1),
                                )
                            dst = (ob1[:, 512 * oc:512 * oc + 512] if last
                                   else ob[:, sti, 512 * oc:512 * oc + 512])
                            use_act = (oc == 0 if final
                                       else drain is not None and (st + oc) % 2 == 0)
                            (drain if use_act and drain is not None
                             else nc.vector.tensor_copy)(dst, pso[:, :])
                            if final:
                                (nc.scalar if oc == 0 else nc.sync).dma_start(
                                    out_d[128 * st:128 * st + 128,
                                          512 * oc:512 * oc + 512],
                                    ob1[:, 512 * oc:512 * oc + 512],
                                )
                        if last and not final:
                            (nc.sync if sti == 0 else nc.scalar).dma_start(
                                out_d[128 * st:128 * st + 128, :], ob1[:, :])
                    if not last:
                        nc.sync.dma_start(
                            out_d[128 * st0:128 * st0 + 256, :].rearrange(
                                "(t p) c -> p t c", p=128),
                            ob[:, :, :],
                        )

            # ---- projection waves with chunk-0/1 attention folded in ----
            emit_qk(wq, qt, 0)
            emit_qk(wk, kt, 0)
            emit_v(0)
            emit_head_pair(0, 0)
            emit_head_pair(0, 1)
            emit_qk(wq, qt, 1)
            emit_qk(wk, kt, 1)
            emit_v(1)
            emit_head_pair(1, 0)
            emit_qk(wq, qt, 2)
            emit_qk(wk, kt, 2)
            emit_v(2)
            emit_qk(wq, qt, 3)
            emit_qk(wk, kt, 3)
            emit_v(3)
            emit_outproj(0, drain=nc.scalar.copy)

            # ---- chunks 3/2/1 interleaved; each outproj is emitted right
            # after the pair it truly depends on (cross-engine waits
            # coalesce to the latest emitted DVE work, so emitting an op
            # after an unrelated pair's normalize stalls it falsely) ----
            emit_head_pair(3, 0)
            emit_head_pair(2, 0)
            emit_head_pair(3, 1)
            s21 = emit_head_pair(2, 1, defer_norm=True)
            emit_outproj(3)          # needs only chunk-3 pairs: runs free
            emit_normalize(s21)
            s11 = emit_head_pair(1, 1, defer_norm=True)
            emit_outproj(2, drain=nc.scalar.copy)   # needs only chunk 2
            emit_normalize(s11)
            emit_outproj(1, drain=nc.scalar.copy, last=True)
    nc.compile()
    return nc


_NC = None


def _get_nc():
    global _NC
    if _NC is None:
        _NC = _build()
    return _NC


def kernel(**inputs):
    x = np.asarray(inputs["inputs"], dtype=np.float32)
    wq = np.asarray(inputs["Wq"], dtype=np.float32)
    wk = np.asarray(inputs["Wk"], dtype=np.float32)
    wv = np.asarray(inputs["Wv"], dtype=np.float32)
    wo = np.asarray(inputs["Wo"], dtype=np.float32)
    bo = np.asarray(inputs["bo"], dtype=np.float32)

    xts = [np.ascontiguousarray(x[b].T).astype(np.float16) for b in range(B)]
    in_maps = []
    for c in range(N_CORES):
        b, g = c // 4, c % 4
        sl = slice(GD * g, GD * g + GD)
        in_maps.append({
            "xt": xts[b],
            "wqT": np.ascontiguousarray(wq[sl, :].T).astype(np.float16),
            "wkT": np.ascontiguousarray(wk[sl, :].T).astype(np.float16),
            "wvT": np.ascontiguousarray(wv[sl, :].T).astype(np.float16),
            "woT": np.ascontiguousarray(wo[:, sl].T).astype(np.float16),
        })

    nc = _get_nc()
    res = run_bass_kernel_spmd(nc, in_maps, core_ids=list(range(N_CORES)))
    out = np.empty((B, S, D), np.float32)
    for b in range(B):
        acc = res.results[4 * b + 0]["out"].astype(np.float32)
        for g in range(1, 4):
            acc = acc + res.results[4 * b + g]["out"].astype(np.float32)
        out[b] = acc + bo
    return out


# revision 68
# speedup vs baseline: 1.0918x; 1.0049x over previous
"""Causal multi-head attention (B=2, S=2048, D=1024, H=16, HD=64) on 8 NeuronCores.

Sharding: core c = 4*b + g handles batch b (2-way data parallel) and head
group g (4-way tensor parallel over the 16 heads, 4 heads per core).
Each core computes its 4 heads' attention plus the partial output
projection (columns of Wo for its heads); the host sums the 4 partials
per batch ("row-parallel" reduction) to produce the full output.

Device layout notes:
  - All device tensors are fp16: matmuls run at the same 1 cycle/row as
    fp32r but DMA bytes halve (5e-4 absmax-relative error in numpy).
  - X is fed transposed (xt = X[b].T, [D,S]) so the d-contraction of the
    QKV projections has d on SBUF partitions.
  - Q,K are produced transposed ([d_head, s]); scores are computed
    transposed (S^T[k,q]) so the P@V matmul needs no transposes at all.
  - V is produced in natural [s, d] layout, augmented with a ones column
    per head so the P@V matmul also yields the softmax denominator.
  - softmax skips max-subtraction (scores/8 ~ N(0,1); exp is safe in f32).
  - Inputs stream in ~17 large DMAs on the SP + Act queues (Pool/gpsimd
    DMAs hang on this stack); output partials are stored fp16, two row
    blocks per DMA.
  - chunks 0 and 1 of the attention run inside the DMA-bound projection
    window (the scalar engine is idle there); the exp-heavy chunks 3 and 2
    run interleaved afterwards.
"""

import numpy as np

import concourse.mybir as mybir
from concourse import bacc
from concourse.tile import TileContext
from concourse.bass_utils import run_bass_kernel_spmd
from concourse.masks import make_upper_triangular

F32 = mybir.dt.float32
F16 = mybir.dt.float16
FP16 = mybir.dt.float16
Exp = mybir.ActivationFunctionType.Exp
Alu = mybir.AluOpType

B, S, D, H, HD = 2, 2048, 1024, 16, 64
GH = 4            # heads per core
GD = GH * HD      # 256 features per core
N_CORES = 8


def _build():
    nc = bacc.Bacc("TRN2", target_bir_lowering=False, name="mha_tp")
    xt_d = nc.dram_tensor("xt", [D, S], F16, kind="ExternalInput")
    wq_d = nc.dram_tensor("wqT", [D, GD], F16, kind="ExternalInput")
    wk_d = nc.dram_tensor("wkT", [D, GD], F16, kind="ExternalInput")
    wv_d = nc.dram_tensor("wvT", [D, GD], F16, kind="ExternalInput")
    wo_d = nc.dram_tensor("woT", [GD, D], F16, kind="ExternalInput")
    out_d = nc.dram_tensor("out", [S, D], F16, kind="ExternalOutput")

    with TileContext(nc) as tc:
        with (
            tc.tile_pool(name="per", bufs=1) as per,
            tc.tile_pool(name="pt", bufs=10) as ptp,
            tc.tile_pool(name="wk1", bufs=2) as wk1,
            tc.tile_pool(name="ob", bufs=6) as obp,
            tc.tile_pool(name="ps_a", bufs=2, space="PSUM") as ps_a,
            tc.tile_pool(name="ps_o", bufs=2, space="PSUM") as ps_o,
            tc.tile_pool(name="ps_c", bufs=2, space="PSUM") as ps_c,
        ):
            xt = per.tile([128, 8, S], F16)        # X^T, d-tile major
            wo = per.tile([128, 2, D], F16)        # Wo^T for our head cols
            qt = per.tile([128, 2, S], FP16)       # Q^T (2 heads per tile)
            kt = per.tile([128, 2, S], FP16)
            vaug = per.tile([128, 16, 4 * (HD + 1)], FP16)  # V + ones col per head
            ctxn = per.tile([128, 2, S], F16)      # normalized ctx^T
            tri = per.tile([128, 128], FP16)       # tri[kk,c]=1 iff kk<=c
            wq = per.tile([128, 8, GD], F16)
            wk = per.tile([128, 8, GD], F16)
            wv = per.tile([128, 8, GD], F16)

            # ---- input DMA: large pieces, SP + Act queues, consumption
            # order: wq, xt0, wv, xt1, wk, xt2, xt3, wo ----
            def ld_x(eng, qlo, qhi, c0, c1):
                eng.dma_start(
                    xt[:, qlo:qhi, c0:c1],
                    xt_d[128 * qlo:128 * qhi, c0:c1].rearrange(
                        "(t p) c -> p t c", p=128),
                )

            def ld_w(eng, w_t, w_d, qlo, qhi):
                eng.dma_start(
                    w_t[:, qlo:qhi, :],
                    w_d[128 * qlo:128 * qhi, :].rearrange(
                        "(t p) c -> p t c", p=128),
                )

            ld_w(nc.sync, wq, wq_d, 0, 2)          # small first pieces
            make_upper_triangular(nc, tri[:, :], val=1.0, diag=True)
            ld_x(nc.scalar, 0, 2, 0, 512)
            ld_w(nc.sync, wq, wq_d, 2, 8)
            ld_x(nc.scalar, 2, 8, 0, 512)
            ld_w(nc.sync, wk, wk_d, 0, 8)          # K0 right after Q0
            ld_w(nc.scalar, wv, wv_d, 0, 8)
            ld_x(nc.sync, 0, 8, 512, 1024)
            ld_x(nc.scalar, 0, 8, 1024, 1536)
            ld_x(nc.sync, 0, 8, 1536, 2048)
            nc.scalar.dma_start(
                wo[:, :, :],
                wo_d[:, :].rearrange("(t p) c -> p t c", p=128),
            )

            def emit_qk(w_t, dst, sc):
                for dp in range(2):
                    ps = ps_a.tile([128, 512], F32, tag="blk")
                    for dt in range(8):
                        nc.tensor.matmul(
                            ps[:, :],
                            w_t[:, dt, 128 * dp:128 * dp + 128],
                            xt[:, dt, 512 * sc:512 * sc + 512],
                            start=(dt == 0), stop=(dt == 7),
                        )
                    nc.vector.tensor_copy(dst[:, dp, 512 * sc:512 * sc + 512], ps[:, :])

            def emit_v(sc):
                for st in range(4 * sc, 4 * sc + 4):
                    psv = ps_a.tile([128, 256], F32, tag="blk")
                    for dt in range(8):
                        nc.tensor.matmul(
                            psv[:, :],
                            xt[:, dt, 128 * st:128 * st + 128],
                            wv[:, dt, :],
                            start=(dt == 0), stop=(dt == 7),
                        )
                    v_dst = vaug[:, st, :].rearrange("p (h c) -> p h c", c=HD + 1)
                    nc.vector.tensor_copy(
                        v_dst[:, :, 0:HD],
                        psv.rearrange("p (h c) -> p h c", c=HD),
                    )
                    # ones column: x*0+1 through DVE so the write is rounded
                    nc.vector.tensor_scalar(
                        v_dst[:, :, HD:HD + 1], psv[:, 0:4], 0.0, 1.0,
                        op0=Alu.mult, op1=Alu.add,
                    )

            def emit_head_pair(qc, i, defer_norm=False):
                """Heads hA=2i (PE rows 0-63) and hB=2i+1 (rows 64-127): their
                score matmuls are emitted alternating so the hardware runs
                them concurrently in disjoint PE row groups."""
                hA, hB = 2 * i, 2 * i + 1
                heads = ((hA, 0), (hB, 64))
                ctxs = {}
                pts = {h: [] for h, _ in heads}
                packs = [
                    (896, ((0, 0, 512), (1, 512, 384))),
                    (384, ((3, 0, 128), (2, 128, 256))),
                ]
                for h, qo in heads:
                    ctx_t = ps_c.tile([65, 512], F32, tag="ctx")
                    ctxs[h] = ctx_t
                # diagonal strips: A and B tiles in flight together, matmuls
                # alternating between the two row groups
                for width, parts in packs:
                    sps = {}
                    for h, qo in heads:
                        sp_t = ps_a.tile([128, 1024], F32, tag="blk")
                        sps[h] = sp_t
                    for j, o, w in parts:
                        k_t = 4 * qc + j
                        for h, qo in heads:
                            nc.tensor.matmul(
                                sps[h][:, o:o + w],
                                kt[qo:qo + 64, i, 128 * k_t:128 * k_t + 128],
                                qt[qo:qo + 64, i, 512 * qc + 128 * j:512 * qc + 128 * j + w],
                                start=True, stop=True,
                            )
                    for h, qo in heads:
                        pt_p = ptp.tile([128, 1024], FP16, tag="pt")
                        nc.scalar.activation(pt_p[:, :width], sps[h][:, :width], Exp, scale=0.125)
                        for ii, (j, o, w) in enumerate(parts):
                            engm = nc.vector if ii == 0 else nc.gpsimd
                            engm.tensor_mul(
                                pt_p[:, o:o + 128], pt_p[:, o:o + 128], tri[:, :]
                            )
                        pts[h].append((pt_p, parts))
                # full blocks (2 k-tiles per tile), pairwise
                for blk in range(2 * qc):
                    sps = {}
                    for h, qo in heads:
                        sp_t = ps_a.tile([128, 1024], F32, tag="blk")
                        sps[h] = sp_t
                    for j2 in range(2):
                        k_t = 2 * blk + j2
                        for h, qo in heads:
                            nc.tensor.matmul(
                                sps[h][:, 512 * j2:512 * j2 + 512],
                                kt[qo:qo + 64, i, 128 * k_t:128 * k_t + 128],
                                qt[qo:qo + 64, i, 512 * qc:512 * qc + 512],
                                start=True, stop=True,
                            )
                    for h, qo in heads:
                        pt_b = ptp.tile([128, 1024], FP16, tag="pt")
                        nc.scalar.activation(pt_b[:, :], sps[h][:, :], Exp, scale=0.125)
                        pts[h].append((pt_b, ((None, 0, 512), (None, 512, 512))))
                # ctx accumulation per head
                for h, qo in heads:
                    ctx = ctxs[h]
                    ctx_mms = []
                    for bi, (pt_t, parts) in enumerate(pts[h]):
                        for pj, (j, o, w) in enumerate(parts):
                            if bi < 2:          # diagonal strip tiles
                                k_t, co = 4 * qc + j, 128 * j
                            else:               # full block tiles
                                k_t, co = 2 * (bi - 2) + pj, 0
                            ctx_mms.append((pt_t, k_t, o, w, co))
                    for n, (pt_t, k_t, o, w, co) in enumerate(ctx_mms):
                        nc.tensor.matmul(
                            ctx[:, co:co + w],
                            vaug[:, k_t, 65 * h:65 * h + 65],
                            pt_t[:, o:o + w],
                            start=(n == 0), stop=(n == len(ctx_mms) - 1),
                        )
                if defer_norm:
                    return (qc, i, heads, ctxs)
                emit_normalize((qc, i, heads, ctxs))
                return None

            def emit_normalize(saved):
                # normalize both heads: l row to SBUF, recip, broadcast,
                # scale each head's PSUM ctx into ctxn
                qc, i, heads, ctxs = saved
                for n, (h, qo) in enumerate(heads):
                    l_sb = wk1.tile([1, 512], F32, tag="lrow")
                    nc.vector.tensor_copy(l_sb[:, :], ctxs[h][64:65, :])
                    r_sb = wk1.tile([1, 512], F32, tag="rrow")
                    nc.vector.reciprocal_approx_fast(r_sb[:, :], l_sb[:, :])
                    rb = wk1.tile([64, 512], F32, tag="rb")
                    nc.gpsimd.partition_broadcast(rb[:, :], r_sb[:1, :], channels=64)
                    nc.vector.tensor_mul(
                        ctxn[qo:qo + 64, i, 512 * qc:512 * qc + 512],
                        ctxs[h][0:64, :], rb[:, :],
                    )

            def emit_outproj(qc, drain=None, last=False):
                # bias is added on the host during the unshard sum; output
                # rows accumulate into fp16 SBUF tiles and ship two row
                # blocks per DMA (HWDGE path). The LAST chunk stores per-st
                # (and the final st per-half) on parallel queues so the
                # end-of-kernel drain+store chain is as short as possible.
                for sp in range(2):
                    st0 = 4 * qc + 2 * sp
                    if not last:
                        ob = obp.tile([128, 2, D], F16, tag="ob", name="ob")
                    for sti in range(2):
                        st = st0 + sti
                        final = last and sp == 1 and sti == 1
                        if last:
                            ob1 = obp.tile([128, D], F16, tag="ob1", name="ob1")
                        for oc in range(2):
                            pso = ps_o.tile([128, 512], F32, tag="po")
                            for dp in range(2):
                                nc.tensor.matmul(
                                    pso[:, :],
                                    ctxn[:, dp, 128 * st:128 * st + 128],
                                    wo[:, dp, 512 * oc:512 * oc + 512],
                                    start=(dp == 0), stop=(dp == 1),
                                )
                            dst = (ob1[:, 512 * oc:512 * oc + 512] if last
                                   else ob[:, sti, 512 * oc:512 * oc + 512])
                            use_act = (oc == 0 if final
                                       else drain is not None and (st + oc) % 2 == 0)
                            (drain if use_act and drain is not None
                             else nc.vector.tensor_copy)(dst, pso[:, :])
                            if final:
                                (nc.scalar if oc == 0 else nc.sync).dma_start(
                                    out_d[128 * st:128 * st + 128,
                                          512 * oc:512 * oc + 512],
                                    ob1[:, 512 * oc:512 * oc + 512],
                                )
                        if last and not final:
                            (nc.sync if sti == 0 else nc.scalar).dma_start(
                                out_d[128 * st:128 * st + 128, :], ob1[:, :])
                    if not last:
                        nc.sync.dma_start(
                            out_d[128 * st0:128 * st0 + 256, :].rearrange(
                                "(t p) c -> p t c", p=128),
                            ob[:, :, :],
                        )

            # ---- projection waves with chunk-0/1 attention folded in ----
            emit_qk(wq, qt, 0)
            emit_qk(wk, kt, 0)
            emit_v(0)
            emit_head_pair(0, 0)
            emit_head_pair(0, 1)
            emit_qk(wq, qt, 1)
            emit_qk(wk, kt, 1)
            emit_v(1)
            emit_head_pair(1, 0)
            emit_qk(wq, qt, 2)
            emit_qk(wk, kt, 2)
            emit_v(2)
            emit_qk(wq, qt, 3)
            emit_qk(wk, kt, 3)
            emit_v(3)
            emit_outproj(0)

            # ---- chunks 3/2/1 interleaved; each outproj is emitted right
            # after the pair it truly depends on (cross-engine waits
            # coalesce to the latest emitted DVE work, so emitting an op
            # after an unrelated pair's normalize stalls it falsely) ----
            emit_head_pair(3, 0)
            emit_head_pair(2, 0)
            emit_head_pair(3, 1)
            s21 = emit_head_pair(2, 1, defer_norm=True)
            emit_outproj(3)          # needs only chunk-3 pairs: runs free
            emit_normalize(s21)
            s11 = emit_head_pair(1, 1, defer_norm=True)
            emit_outproj(2, drain=nc.scalar.copy)   # needs only chunk 2
            emit_normalize(s11)
            emit_outproj(1, drain=nc.scalar.copy, last=True)
    nc.compile()
    return nc


_NC = None


def _get_nc():
    global _NC
    if _NC is None:
        _NC = _build()
    return _NC


def kernel(**inputs):
    x = np.asarray(inputs["inputs"], dtype=np.float32)
    wq = np.asarray(inputs["Wq"], dtype=np.float32)
    wk = np.asarray(inputs["Wk"], dtype=np.float32)
    wv = np.asarray(inputs["Wv"], dtype=np.float32)
    wo = np.asarray(inputs["Wo"], dtype=np.float32)
    bo = np.asarray(inputs["bo"], dtype=np.float32)

    xts = [np.ascontiguousarray(x[b].T).astype(np.float16) for b in range(B)]
    in_maps = []
    for c in range(N_CORES):
        b, g = c // 4, c % 4
        sl = slice(GD * g, GD * g + GD)
        in_maps.append({
            "xt": xts[b],
            "wqT": np.ascontiguousarray(wq[sl, :].T).astype(np.float16),
            "wkT": np.ascontiguousarray(wk[sl, :].T).astype(np.float16),
            "wvT": np.ascontiguousarray(wv[sl, :].T).astype(np.float16),
            "woT": np.ascontiguousarray(wo[:, sl].T).astype(np.float16),
        })

    nc = _get_nc()
    res = run_bass_kernel_spmd(nc, in_maps, core_ids=list(range(N_CORES)))
    out = np.empty((B, S, D), np.float32)
    for b in range(B):
        acc = res.results[4 * b + 0]["out"].astype(np.float32)
        for g in range(1, 4):
            acc = acc + res.results[4 * b + g]["out"].astype(np.float32)
        out[b] = acc + bo
    return out


# revision 70
# speedup vs baseline: 1.0935x; 1.0015x over previous
"""Causal multi-head attention (B=2, S=2048, D=1024, H=16, HD=64) on 8 NeuronCores.

Sharding: core c = 4*b + g handles batch b (2-way data parallel) and head
group g (4-way tensor parallel over the 16 heads, 4 heads per core).
Each core computes its 4 heads' attention plus the partial output
projection (columns of Wo for its heads); the host sums the 4 partials
per batch ("row-parallel" reduction) to produce the full output.

Device layout notes:
  - All device tensors are fp16: matmuls run at the same 1 cycle/row as
    fp32r but DMA bytes halve (5e-4 absmax-relative error in numpy).
  - X is fed transposed (xt = X[b].T, [D,S]) so the d-contraction of the
    QKV projections has d on SBUF partitions.
  - Q,K are produced transposed ([d_head, s]); scores are computed
    transposed (S^T[k,q]) so the P@V matmul needs no transposes at all.
  - V is produced in natural [s, d] layout, augmented with a ones column
    per head so the P@V matmul also yields the softmax denominator.
  - softmax skips max-subtraction (scores/8 ~ N(0,1); exp is safe in f32).
  - Inputs stream in ~17 large DMAs on the SP + Act queues (Pool/gpsimd
    DMAs hang on this stack); output partials are stored fp16, two row
    blocks per DMA.
  - chunks 0 and 1 of the attention run inside the DMA-bound projection
    window (the scalar engine is idle there); the exp-heavy chunks 3 and 2
    run interleaved afterwards.
"""

import numpy as np

import concourse.mybir as mybir
from concourse import bacc
from concourse.tile import TileContext
from concourse.bass_utils import run_bass_kernel_spmd
from concourse.masks import make_upper_triangular

F32 = mybir.dt.float32
F16 = mybir.dt.float16
FP16 = mybir.dt.float16
Exp = mybir.ActivationFunctionType.Exp
Alu = mybir.AluOpType

B, S, D, H, HD = 2, 2048, 1024, 16, 64
GH = 4            # heads per core
GD = GH * HD      # 256 features per core
N_CORES = 8


def _build():
    nc = bacc.Bacc("TRN2", target_bir_lowering=False, name="mha_tp")
    xt_d = nc.dram_tensor("xt", [D, S], F16, kind="ExternalInput")
    wq_d = nc.dram_tensor("wqT", [D, GD], F16, kind="ExternalInput")
    wk_d = nc.dram_tensor("wkT", [D, GD], F16, kind="ExternalInput")
    wv_d = nc.dram_tensor("wvT", [D, GD], F16, kind="ExternalInput")
    wo_d = nc.dram_tensor("woT", [GD, D], F16, kind="ExternalInput")
    out_d = nc.dram_tensor("out", [S, D], F16, kind="ExternalOutput")

    with TileContext(nc) as tc:
        with (
            tc.tile_pool(name="per", bufs=1) as per,
            tc.tile_pool(name="pt", bufs=10) as ptp,
            tc.tile_pool(name="wk1", bufs=2) as wk1,
            tc.tile_pool(name="ob", bufs=6) as obp,
            tc.tile_pool(name="ps_a", bufs=2, space="PSUM") as ps_a,
            tc.tile_pool(name="ps_o", bufs=2, space="PSUM") as ps_o,
            tc.tile_pool(name="ps_c", bufs=2, space="PSUM") as ps_c,
        ):
            xt = per.tile([128, 8, S], F16)        # X^T, d-tile major
            wo = per.tile([128, 2, D], F16)        # Wo^T for our head cols
            qt = per.tile([128, 2, S], FP16)       # Q^T (2 heads per tile)
            kt = per.tile([128, 2, S], FP16)
            vaug = per.tile([128, 16, 4 * (HD + 1)], FP16)  # V + ones col per head
            ctxn = per.tile([128, 2, S], F16)      # normalized ctx^T
            tri = per.tile([128, 128], FP16)       # tri[kk,c]=1 iff kk<=c
            wq = per.tile([128, 8, GD], F16)
            wk = per.tile([128, 8, GD], F16)
            wv = per.tile([128, 8, GD], F16)

            # ---- input DMA: large pieces, SP + Act queues, consumption
            # order: wq, xt0, wv, xt1, wk, xt2, xt3, wo ----
            def ld_x(eng, qlo, qhi, c0, c1):
                eng.dma_start(
                    xt[:, qlo:qhi, c0:c1],
                    xt_d[128 * qlo:128 * qhi, c0:c1].rearrange(
                        "(t p) c -> p t c", p=128),
                )

            def ld_w(eng, w_t, w_d, qlo, qhi):
                eng.dma_start(
                    w_t[:, qlo:qhi, :],
                    w_d[128 * qlo:128 * qhi, :].rearrange(
                        "(t p) c -> p t c", p=128),
                )

            ld_w(nc.sync, wq, wq_d, 0, 2)          # small first pieces
            make_upper_triangular(nc, tri[:, :], val=1.0, diag=True)
            ld_x(nc.scalar, 0, 2, 0, 512)
            ld_w(nc.sync, wq, wq_d, 2, 8)
            ld_x(nc.scalar, 2, 8, 0, 512)
            ld_w(nc.sync, wk, wk_d, 0, 8)          # K0 right after Q0
            ld_w(nc.scalar, wv, wv_d, 0, 8)
            ld_x(nc.sync, 0, 8, 512, 1024)
            ld_x(nc.scalar, 0, 8, 1024, 1536)
            ld_x(nc.sync, 0, 8, 1536, 2048)
            nc.scalar.dma_start(
                wo[:, :, :],
                wo_d[:, :].rearrange("(t p) c -> p t c", p=128),
            )

            def emit_qk(w_t, dst, sc):
                for dp in range(2):
                    ps = ps_a.tile([128, 512], F32, tag="blk")
                    for dt in range(8):
                        nc.tensor.matmul(
                            ps[:, :],
                            w_t[:, dt, 128 * dp:128 * dp + 128],
                            xt[:, dt, 512 * sc:512 * sc + 512],
                            start=(dt == 0), stop=(dt == 7),
                        )
                    nc.vector.tensor_copy(dst[:, dp, 512 * sc:512 * sc + 512], ps[:, :])

            def emit_v(sc):
                for st in range(4 * sc, 4 * sc + 4):
                    psv = ps_a.tile([128, 256], F32, tag="blk")
                    for dt in range(8):
                        nc.tensor.matmul(
                            psv[:, :],
                            xt[:, dt, 128 * st:128 * st + 128],
                            wv[:, dt, :],
                            start=(dt == 0), stop=(dt == 7),
                        )
                    v_dst = vaug[:, st, :].rearrange("p (h c) -> p h c", c=HD + 1)
                    nc.vector.tensor_copy(
                        v_dst[:, :, 0:HD],
                        psv.rearrange("p (h c) -> p h c", c=HD),
                    )
                    # ones column: x*0+1 through DVE so the write is rounded
                    nc.vector.tensor_scalar(
                        v_dst[:, :, HD:HD + 1], psv[:, 0:4], 0.0, 1.0,
                        op0=Alu.mult, op1=Alu.add,
                    )

            def emit_head_pair(qc, i, defer_norm=False):
                """Heads hA=2i (PE rows 0-63) and hB=2i+1 (rows 64-127): their
                score matmuls are emitted alternating so the hardware runs
                them concurrently in disjoint PE row groups."""
                hA, hB = 2 * i, 2 * i + 1
                heads = ((hA, 0), (hB, 64))
                ctxs = {}
                pts = {h: [] for h, _ in heads}
                packs = [
                    (896, ((0, 0, 512), (1, 512, 384))),
                    (384, ((3, 0, 128), (2, 128, 256))),
                ]
                for h, qo in heads:
                    ctx_t = ps_c.tile([65, 512], F32, tag="ctx")
                    ctxs[h] = ctx_t
                # diagonal strips: A and B tiles in flight together, matmuls
                # alternating between the two row groups
                for width, parts in packs:
                    sps = {}
                    for h, qo in heads:
                        sp_t = ps_a.tile([128, 1024], F32, tag="blk")
                        sps[h] = sp_t
                    for j, o, w in parts:
                        k_t = 4 * qc + j
                        for h, qo in heads:
                            nc.tensor.matmul(
                                sps[h][:, o:o + w],
                                kt[qo:qo + 64, i, 128 * k_t:128 * k_t + 128],
                                qt[qo:qo + 64, i, 512 * qc + 128 * j:512 * qc + 128 * j + w],
                                start=True, stop=True,
                            )
                    for h, qo in heads:
                        pt_p = ptp.tile([128, 1024], FP16, tag="pt")
                        nc.scalar.activation(pt_p[:, :width], sps[h][:, :width], Exp, scale=0.125)
                        for ii, (j, o, w) in enumerate(parts):
                            engm = nc.vector if ii == 0 else nc.gpsimd
                            engm.tensor_mul(
                                pt_p[:, o:o + 128], pt_p[:, o:o + 128], tri[:, :]
                            )
                        pts[h].append((pt_p, parts))
                # full blocks (2 k-tiles per tile), pairwise
                for blk in range(2 * qc):
                    sps = {}
                    for h, qo in heads:
                        sp_t = ps_a.tile([128, 1024], F32, tag="blk")
                        sps[h] = sp_t
                    for j2 in range(2):
                        k_t = 2 * blk + j2
                        for h, qo in heads:
                            nc.tensor.matmul(
                                sps[h][:, 512 * j2:512 * j2 + 512],
                                kt[qo:qo + 64, i, 128 * k_t:128 * k_t + 128],
                                qt[qo:qo + 64, i, 512 * qc:512 * qc + 512],
                                start=True, stop=True,
                            )
                    for h, qo in heads:
                        pt_b = ptp.tile([128, 1024], FP16, tag="pt")
                        nc.scalar.activation(pt_b[:, :], sps[h][:, :], Exp, scale=0.125)
                        pts[h].append((pt_b, ((None, 0, 512), (None, 512, 512))))
                # ctx accumulation per head
                for h, qo in heads:
                    ctx = ctxs[h]
                    ctx_mms = []
                    for bi, (pt_t, parts) in enumerate(pts[h]):
                        for pj, (j, o, w) in enumerate(parts):
                            if bi < 2:          # diagonal strip tiles
                                k_t, co = 4 * qc + j, 128 * j
                            else:               # full block tiles
                                k_t, co = 2 * (bi - 2) + pj, 0
                            ctx_mms.append((pt_t, k_t, o, w, co))
                    for n, (pt_t, k_t, o, w, co) in enumerate(ctx_mms):
                        nc.tensor.matmul(
                            ctx[:, co:co + w],
                            vaug[:, k_t, 65 * h:65 * h + 65],
                            pt_t[:, o:o + w],
                            start=(n == 0), stop=(n == len(ctx_mms) - 1),
                        )
                if defer_norm:
                    return (qc, i, heads, ctxs)
                emit_normalize((qc, i, heads, ctxs))
                return None

            def emit_normalize(saved, act_copy=False):
                # normalize both heads: l row to SBUF, recip, broadcast,
                # scale each head's PSUM ctx into ctxn
                qc, i, heads, ctxs = saved
                for n, (h, qo) in enumerate(heads):
                    l_sb = wk1.tile([1, 512], F32, tag="lrow")
                    (nc.scalar.copy if act_copy else nc.vector.tensor_copy)(
                        l_sb[:, :], ctxs[h][64:65, :])
                    r_sb = wk1.tile([1, 512], F32, tag="rrow")
                    nc.vector.reciprocal_approx_fast(r_sb[:, :], l_sb[:, :])
                    rb = wk1.tile([64, 512], F32, tag="rb")
                    nc.gpsimd.partition_broadcast(rb[:, :], r_sb[:1, :], channels=64)
                    nc.vector.tensor_mul(
                        ctxn[qo:qo + 64, i, 512 * qc:512 * qc + 512],
                        ctxs[h][0:64, :], rb[:, :],
                    )

            def emit_outproj(qc, drain=None, last=False):
                # bias is added on the host during the unshard sum; output
                # rows accumulate into fp16 SBUF tiles and ship two row
                # blocks per DMA (HWDGE path). The LAST chunk stores per-st
                # (and the final st per-half) on parallel queues so the
                # end-of-kernel drain+store chain is as short as possible.
                for sp in range(2):
                    st0 = 4 * qc + 2 * sp
                    if not last:
                        ob = obp.tile([128, 2, D], F16, tag="ob", name="ob")
                    for sti in range(2):
                        st = st0 + sti
                        final = last and sp == 1 and sti == 1
                        if last:
                            ob1 = obp.tile([128, D], F16, tag="ob1", name="ob1")
                        for oc in range(2):
                            pso = ps_o.tile([128, 512], F32, tag="po")
                            for dp in range(2):
                                nc.tensor.matmul(
                                    pso[:, :],
                                    ctxn[:, dp, 128 * st:128 * st + 128],
                                    wo[:, dp, 512 * oc:512 * oc + 512],
                                    start=(dp == 0), stop=(dp == 1),
                                )
                            dst = (ob1[:, 512 * oc:512 * oc + 512] if last
                                   else ob[:, sti, 512 * oc:512 * oc + 512])
                            use_act = (oc == 0 if final
                                       else drain is not None and (st + oc) % 2 == 0)
                            (drain if use_act and drain is not None
                             else nc.vector.tensor_copy)(dst, pso[:, :])
                            if final:
                                (nc.scalar if oc == 0 else nc.sync).dma_start(
                                    out_d[128 * st:128 * st + 128,
                                          512 * oc:512 * oc + 512],
                                    ob1[:, 512 * oc:512 * oc + 512],
                                )
                        if last and not final:
                            (nc.sync if sti == 0 else nc.scalar).dma_start(
                                out_d[128 * st:128 * st + 128, :], ob1[:, :])
                    if not last:
                        nc.sync.dma_start(
                            out_d[128 * st0:128 * st0 + 256, :].rearrange(
                                "(t p) c -> p t c", p=128),
                            ob[:, :, :],
                        )

            # ---- projection waves with chunk-0/1 attention folded in ----
            emit_qk(wq, qt, 0)
            emit_qk(wk, kt, 0)
            emit_v(0)
            emit_head_pair(0, 0)
            emit_head_pair(0, 1)
            emit_qk(wq, qt, 1)
            emit_qk(wk, kt, 1)
            emit_v(1)
            emit_head_pair(1, 0)
            emit_qk(wq, qt, 2)
            emit_qk(wk, kt, 2)
            emit_v(2)
            emit_qk(wq, qt, 3)
            emit_qk(wk, kt, 3)
            emit_v(3)
            emit_outproj(0)

            # ---- chunks 3/2/1 interleaved; each outproj is emitted right
            # after the pair it truly depends on (cross-engine waits
            # coalesce to the latest emitted DVE work, so emitting an op
            # after an unrelated pair's normalize stalls it falsely) ----
            emit_head_pair(3, 0)
            emit_head_pair(2, 0)
            emit_head_pair(3, 1)
            s21 = emit_head_pair(2, 1, defer_norm=True)
            emit_outproj(3)          # needs only chunk-3 pairs: runs free
            emit_normalize(s21)
            s11 = emit_head_pair(1, 1, defer_norm=True)
            emit_outproj(2, drain=nc.scalar.copy)   # needs only chunk 2
            emit_normalize(s11, act_copy=True)
            emit_outproj(1, drain=nc.scalar.copy, last=True)
    nc.compile()
    return nc


_NC = None


def _get_nc():
    global _NC
    if _NC is None:
        _NC = _build()
    return _NC


def kernel(**inputs):
    x = np.asarray(inputs["inputs"], dtype=np.float32)
    wq = np.asarray(inputs["Wq"], dtype=np.float32)
    wk = np.asarray(inputs["Wk"], dtype=np.float32)
    wv = np.asarray(inputs["Wv"], dtype=np.float32)
    wo = np.asarray(inputs["Wo"], dtype=np.float32)
    bo = np.asarray(inputs["bo"], dtype=np.float32)

    xts = [np.ascontiguousarray(x[b].T).astype(np.float16) for b in range(B)]
    in_maps = []
    for c in range(N_CORES):
        b, g = c // 4, c % 4
        sl = slice(GD * g, GD * g + GD)
        in_maps.append({
            "xt": xts[b],
            "wqT": np.ascontiguousarray(wq[sl, :].T).astype(np.float16),
            "wkT": np.ascontiguousarray(wk[sl, :].T).astype(np.float16),
            "wvT": np.ascontiguousarray(wv[sl, :].T).astype(np.float16),
            "woT": np.ascontiguousarray(wo[:, sl].T).astype(np.float16),
        })

    nc = _get_nc()
    res = run_bass_kernel_spmd(nc, in_maps, core_ids=list(range(N_CORES)))
    out = np.empty((B, S, D), np.float32)
    for b in range(B):
        acc = res.results[4 * b + 0]["out"].astype(np.float32)
        for g in range(1, 4):
            acc = acc + res.results[4 * b + g]["out"].astype(np.float32)
        out[b] = acc + bo
    return out
